# revision 44
# baseline (speedup 1.0000x reference)
"""MoSARA MoE-routing kernel for 8 Trainium2 NeuronCores.

Math: the reference materializes per-expert delta weights
    delta_W[e] = U_k @ diag(lambda_k[e]) @ V_k,  out = sum_e g[b,e] * x @ (W+delta_W[e]).T
but since softmax gates sum to 1 this collapses to
    out = (x @ W.T + ((x @ V_k.T) * (g @ lambda_k)) @ U_k.T) * (1+v)
with g = softmax_e((x @ U_k @ router_W1) * router_W2[e]).

Final version (112.8us baseline -> ~95us measured):
  - fp8 DoubleRow for the low-rank terms: phase 1 (s = x@V.T, s1 = x@u1)
    in fp8e4 DR (256-deep contraction per MM, half the matmuls); V is
    pre-scaled by 32 (entries ~0.7 in fp8 range), the 1/32 folded into
    lambda.  The correction z @ U.T runs in fp8e5 DR at natural scale
    (z ~ 0.02, U ~ 0.02 are e5m2-normal) so it accumulates straight
    into the W-term PSUM groups - no combine op, no extra banks.
  - ONE consumption-ordered Sync-HWDGE input stream (the ~12.6MB
    aggregate read is HBM-bandwidth-bound at ~300-330GB/s/core; any
    queue split or misordering starves the PE and re-throttles the HAM
    clock): c8 fp8 bundle [x8|32V|u1] first (paces phase 1), then
    per-dc merged [xT|W.T] bf16 chunks, ut8 last.  Output DMAs on the
    otherwise-idle Scalar queue.
  - 11 junk warm-up matmuls on memset tiles spin the PE HAM clock gate
    to 2.4GHz while the first DMA (~4.5us latency) lands.
  - each bc block is self-contained: 16x4 W-matmuls + U-DR matmuls
    appended at dc14/15 (PSUM accumulation order is free), per-ni
    psum->bf16 casts alternate Vector/Scalar engines, DMA out bf16
    (host upcasts to fp32).
  - softmax gating runs inside bc0's W-loop with the row-max shift
    folded into the router weights: logits' = (w2-a)*s1 - Relu((b-a)*s1)
    (a=max w2, b=min w2) - the shift term computes on the Scalar engine
    straight from the s1 psum, in parallel with the s1row copy on
    Vector; all tiny gate matmuls use bf16 operands (an fp32 matmul
    costs 2 half-rate passes) and are placed 1-2 dc after their
    cross-engine producers so the PE never waits.
"""

import numpy as np
import ml_dtypes

import concourse.mybir as mybir
import concourse.tile as tile
from concourse import bacc
from concourse.bass_utils import run_bass_kernel_spmd

B, D, K, E = 4096, 2048, 512, 8
N_CORES = 8
BS = B // N_CORES          # 512 tokens per core
P = 128
ND = D // P                # 16 d-chunks
NJ = ND // 2               # 8 d-pair chunks (DoubleRow)
NK = K // P                # 4 k-chunks
NN = D // 512              # 4 n-chunks of 512
NB = BS // P               # 4 b-chunks per core
C8W = BS + K + 16          # 1040 cols: [x8 | 32*V | u1pad]

BF16 = mybir.dt.bfloat16
F32 = mybir.dt.float32
F8E4 = mybir.dt.float8e4
F8E5 = mybir.dt.float8e5
DR = mybir.MatmulPerfMode.DoubleRow

_PROG = None


def _emit(tc, nc, c8d, xwd, ut8d, lamd, w2cd, nabd, outd):
    from contextlib import ExitStack

    with ExitStack() as ctx:
        const = ctx.enter_context(tc.tile_pool(name="const", bufs=1))
        xpool = ctx.enter_context(tc.tile_pool(name="xpool", bufs=1))
        wpool = ctx.enter_context(tc.tile_pool(name="wpool", bufs=1))
        work = ctx.enter_context(tc.tile_pool(name="work", bufs=1))
        opool = ctx.enter_context(tc.tile_pool(name="opool", bufs=2))
        ps = ctx.enter_context(tc.tile_pool(name="ps", bufs=8, space="PSUM"))

        # warm-up operands + small constants (memsets split across engines so
        # both land right after the preamble barrier)
        wu_w = const.tile([P, P], BF16, tag="wu_w")
        nc.vector.memset(wu_w[:], 0.125)
        wu_x = const.tile([P, 512], BF16, tag="wu_x")
        nc.gpsimd.memset(wu_x[:], 0.125)
        lam_sb = const.tile([E, K], BF16, tag="lam")
        nc.gpsimd.dma_start(out=lam_sb[:], in_=lamd[:])
        w2c_sb = const.tile([1, E], BF16, tag="w2c")
        nc.gpsimd.dma_start(out=w2c_sb[:], in_=w2cd[:])
        nab_sb = const.tile([1, 1], F32, tag="nab")
        nc.gpsimd.dma_start(out=nab_sb[:], in_=nabd[:])
        ones8 = const.tile([E, 1], BF16, tag="ones8")
        nc.vector.memset(ones8[:], 1.0)
        ones18 = const.tile([1, E], BF16, tag="ones18")
        nc.vector.memset(ones18[:], 1.0)
        m1s18 = const.tile([1, E], BF16, tag="m1s18")
        nc.vector.memset(m1s18[:], -1.0)

        # ---- input stream: ONE Sync HWDGE queue in exact consumption order.
        # The aggregate HBM read (~12.6MB) is bandwidth-bound; any queue
        # split or misordering starves the PE (v2 lost ~20us to wt-starved
        # W-blocks + HAM re-throttle).  [x|W] merge keeps per-dc arrival
        # granularity with half the DMA triggers.
        c8 = xpool.tile([P, ND, C8W], F8E4, tag="c8", name="c8")
        xws = [xpool.tile([P, BS + D], BF16, tag=f"xw{dc}", name=f"xw{dc}")
               for dc in range(ND)]

        def dma_c8(lo, hi):
            nc.sync.dma_start(out=c8[:, lo:hi, :], in_=c8d[:, lo:hi, :])

        def dma_xw(dc):
            nc.sync.dma_start(out=xws[dc][:], in_=xwd[dc * P:(dc + 1) * P, :])

        # small head chunk so phase 1 starts as early as the DMA latency
        # allows; all of c8 lands before the first xw chunks are needed.
        # Everything on ONE Sync queue in consumption order: measured three
        # times (v2/v6/v12) that any second-queue transfer at the start
        # steals HBM bandwidth from the stream the PE is pacing on.
        dma_c8(0, 2)
        dma_c8(2, 4)
        dma_c8(4, 8)
        dma_c8(8, 12)
        dma_c8(12, 16)
        for dc in range(ND):
            dma_xw(dc)
        ut8 = wpool.tile([P, NK, D], F8E5, tag="ut8", name="ut8")
        for t in range(2):
            nc.sync.dma_start(out=ut8[:, 2 * t:2 * t + 2, :],
                              in_=ut8d[:, 2 * t:2 * t + 2, :])

        # ---- phase 1 PSUM group (warm-up junk matmuls reuse sps[0] so only
        # 5 of the 8 banks are held before the main pass) ----
        s1_ps = ps.tile([16, BS], F32, tag="ps", name="s1_ps")
        sps = [ps.tile([P, BS], F32, tag="ps", name=f"sp{kc}") for kc in range(NK)]

        # PE warm-up: junk matmuls while the first DMAs land (the HAM
        # clock-gate needs ~3.4us of PE activity to reach 2.4GHz, and the
        # first c8 chunk takes ~4.5us to arrive)
        for _ in range(11):
            nc.tensor.matmul(sps[0][:], wu_w[:], wu_x[:], start=True, stop=True)
        for j in range(NJ):
            pair = c8[:, 2 * j:2 * j + 2, :]
            for kc in range(NK):
                nc.tensor.matmul(sps[kc][:],
                                 pair[:, :, BS + kc * P:BS + (kc + 1) * P],
                                 pair[:, :, 0:BS],
                                 start=(j == 0), stop=(j == NJ - 1), perf_mode=DR)
            nc.tensor.matmul(s1_ps[:], pair[:, :, BS + K:BS + K + 16],
                             pair[:, :, 0:BS],
                             start=(j == 0), stop=(j == NJ - 1), perf_mode=DR)

        # exact-softmax row-max shift folded into the router weights:
        # logits'[e] = (w2[e]-a)*s1 + min((a-b)*s1, 0)  with a=max(w2),
        # b=min(w2) equals logits[e]-max_e(logits).  min(y,0) = -Relu(-y),
        # so the shift term computes on the SCALAR engine straight from the
        # s1 psum (parallel with the s1row copy on Vector), and the minus
        # sign rides the -1 weights of the second e_ps matmul.
        s1row = work.tile([1, BS], BF16, tag="s1row")
        mneg_r = work.tile([1, BS], BF16, tag="mneg_r")
        nc.vector.tensor_copy(s1row[:], s1_ps[0:1, :])
        nc.scalar.activation(mneg_r[:], s1_ps[0:1, :],
                             mybir.ActivationFunctionType.Relu,
                             scale=nab_sb[:, 0:1])
        # s_sb copies run on the Scalar engine (ACT reads PSUM) so the
        # Vector queue stays clear for the gating chain
        s_sb = [work.tile([P, BS], F32, tag=f"s{kc}", name=f"s{kc}")
                for kc in range(NK)]

        # SBUF staging for the gating chain (filled while bc0 W-matmuls run)
        g_sb = work.tile([E, BS], BF16, tag="g")
        rden = work.tile([1, BS], F32, tag="rden")
        rden_bf = work.tile([1, BS], BF16, tag="rden_bf")
        gn_sb = work.tile([E, BS], BF16, tag="gn")
        # z8[jj][:, i, :] holds z for kc = 2*jj + i, e5m2 at natural scale
        z8 = [work.tile([P, 2, BS], F8E5, tag=f"z8{jj}", name=f"z8{jj}")
              for jj in range(2)]

        def emit_lam_z(kc, pstate):
            lp = ps.tile([P, BS], F32, tag="ps", name=f"lp{kc}")
            nc.tensor.matmul(lp[:], lam_sb[:, kc * P:(kc + 1) * P],
                             gn_sb[:], start=True, stop=True)
            nc.vector.tensor_tensor(z8[kc // 2][:, kc % 2, :], s_sb[kc][:], lp[:],
                                    mybir.AluOpType.mult)

        def emit_gate_mm(step, pstate):
            # tiny router matmuls spread through bc0's W-loop; their ACT/DVE
            # producers run in the shadow of the surrounding big matmuls
            if step == 0:
                e_ps = ps.tile([E, BS], F32, tag="ps", name="e_ps")
                nc.tensor.matmul(e_ps[:], w2c_sb[:], s1row[:], start=True, stop=False)
                nc.tensor.matmul(e_ps[:], m1s18[:], mneg_r[:], start=False, stop=True)
                pstate["e_ps"] = e_ps
            elif step == 1:
                nc.scalar.activation(g_sb[:], pstate["e_ps"][:],
                                     mybir.ActivationFunctionType.Exp)
            elif step == 2:
                den_ps = ps.tile([1, BS], F32, tag="ps", name="den_ps")
                nc.tensor.matmul(den_ps[:], ones8[:], g_sb[:], start=True, stop=True)
                pstate["den_ps"] = den_ps
            elif step == 3:
                nc.vector.reciprocal_approx_fast(out=rden[:],
                                                 in_=pstate["den_ps"][0:1, :])
                nc.vector.tensor_copy(rden_bf[:], rden[:])
            elif step == 4:
                # bf16 operands: an fp32 matmul here costs 2 half-rate passes
                r8_ps = ps.tile([E, BS], F32, tag="ps", name="r8_ps")
                nc.tensor.matmul(r8_ps[:], ones18[:], rden_bf[:], start=True, stop=True)
                pstate["r8_ps"] = r8_ps
            elif step == 5:
                nc.vector.tensor_tensor(gn_sb[:], g_sb[:], pstate["r8_ps"][:],
                                        mybir.AluOpType.mult)

        # ---- main pass: out = x @ W'.T + z8 @ U'.T.  Each bc block is
        # self-contained: 16x4 W-matmuls with the U DoubleRow matmuls
        # appended at dc14/15 (PSUM accumulation order is free), then the
        # per-ni cast+DMA overlap the next block.  bc0 carries the gating
        # chain (spread 2 dc apart so each tiny matmul's cross-engine
        # producer hides under the W stream) and the lam/z8 production.
        # gate step0 can only run once the s1row+mneg DVE chain (~1.5us
        # after phase-1 stop) completes.  Steps 1/3/5 are ACT/DVE-only;
        # copy_at stages the s_sb psum->sbuf copies on the Scalar queue
        # after the Exp.
        pstate = {}
        gate_at = {2: 0, 3: 1, 6: 2, 7: 3, 10: 4, 11: 5}
        lam_at = {12: [0], 13: [1], 14: [2], 15: [3]}
        copy_at = {4: 0, 5: 1, 6: 2, 7: 3}
        # bc0's dc10-15 are xw-arrival-bound (~2us of PE idle).  bc1's
        # first INTER chunks use long-resident data, so pre-consuming
        # them (ni0-2; ni3's bank frees later) inside bc0's tail both
        # fills the idle and removes work from the post-arrival critical
        # path: END = arrival(xw15) + everything emitted after bc0-dc15.
        INTER = 6
        blk_psums = {}

        def get_psums(bc):
            if bc not in blk_psums:
                blk_psums[bc] = [ps.tile([P, 512], F32, tag="ps",
                                         name=f"po{bc}_{i}") for i in range(NN)]
            return blk_psums[bc]

        def emit_u_mm(psums, bc, ni, jj):
            nc.tensor.matmul(psums[ni][:],
                             z8[jj][:, :, bc * P:(bc + 1) * P],
                             ut8[:, 2 * jj:2 * jj + 2, ni * 512:(ni + 1) * 512],
                             start=False, stop=(jj == 1), perf_mode=DR)

        def emit_block(bc):
            psums = get_psums(bc)
            # bc0-ni3's psum bank is the one phase 1's s1 group vacates;
            # deferring its first two chunks past dc2 hides the bank-free
            # wait (s1row/Relu readers finish ~2 dc into the block)
            defer = {3: 2} if bc == 0 else {}
            for dc in range(ND):
                lhs = xws[dc][:, bc * P:(bc + 1) * P]
                for ni in range(NN):
                    d0 = defer.get(ni, 0)
                    if dc < d0:
                        continue
                    if bc == 1 and ni < 3 and dc < INTER:
                        continue  # pre-consumed in bc0's tail
                    nc.tensor.matmul(psums[ni][:], lhs,
                                     xws[dc][:, BS + ni * 512:BS + (ni + 1) * 512],
                                     start=(dc == d0), stop=False)
                if bc == 0 and dc in gate_at:
                    emit_gate_mm(gate_at[dc], pstate)
                if bc == 0 and dc in copy_at:
                    kc = copy_at[dc]
                    nc.scalar.activation(s_sb[kc][:], sps[kc][:],
                                         mybir.ActivationFunctionType.Copy)
                if bc == 0 and dc in lam_at:
                    for kc in lam_at[dc]:
                        emit_lam_z(kc, pstate)
                if bc == 0 and dc >= ND - INTER:
                    j = dc - (ND - INTER)
                    nx = get_psums(1)
                    for ni in range(3):
                        nc.tensor.matmul(nx[ni][:], xws[j][:, P:2 * P],
                                         xws[j][:, BS + ni * 512:BS + (ni + 1) * 512],
                                         start=(j == 0), stop=False)
            for ni, d0 in defer.items():
                for dc in range(d0):
                    nc.tensor.matmul(psums[ni][:],
                                     xws[dc][:, bc * P:(bc + 1) * P],
                                     xws[dc][:, BS + ni * 512:BS + (ni + 1) * 512],
                                     start=False, stop=False)
            # both U DoubleRow batches contiguous at the block end: each
            # bf16->DR mode switch pays ~0.2us (DR disables fast weight
            # load), so enter DR once per block, not twice
            for jj in range(2):
                for ni in range(NN):
                    emit_u_mm(psums, bc, ni, jj)
            o_sb = opool.tile([P, D], BF16, tag="o", name=f"o{bc}")
            for ni in range(NN):
                # alternate cast engines and DMA queues so the final
                # block's drains pipeline two-wide
                osl = o_sb[:, ni * 512:(ni + 1) * 512]
                odst = outd[bc * P:(bc + 1) * P, ni * 512:(ni + 1) * 512]
                if ni % 2 == 0:
                    nc.vector.tensor_copy(osl, psums[ni][:])
                    nc.sync.dma_start(out=odst, in_=osl)
                else:
                    nc.scalar.activation(osl, psums[ni][:],
                                         mybir.ActivationFunctionType.Copy)
                    nc.scalar.dma_start(out=odst, in_=osl)

        for bc in range(NB):
            emit_block(bc)


def build_program():
    nc = bacc.Bacc("TRN2", target_bir_lowering=False, debug=False)
    c8d = nc.dram_tensor("c8", (P, ND, C8W), F8E4, kind="ExternalInput").ap()
    xwd = nc.dram_tensor("xw", (D, BS + D), BF16, kind="ExternalInput").ap()
    ut8d = nc.dram_tensor("ut8", (P, NK, D), F8E5, kind="ExternalInput").ap()
    lamd = nc.dram_tensor("lam", (E, K), BF16, kind="ExternalInput").ap()
    w2cd = nc.dram_tensor("w2c", (1, E), BF16, kind="ExternalInput").ap()
    nabd = nc.dram_tensor("nab", (1, 1), F32, kind="ExternalInput").ap()
    outd = nc.dram_tensor("out", (BS, D), BF16, kind="ExternalOutput").ap()

    with tile.TileContext(nc) as tc:
        _emit(tc, nc, c8d, xwd, ut8d, lamd, w2cd, nabd, outd)
    nc.compile()
    return nc


def _get_prog():
    global _PROG
    if _PROG is None:
        _PROG = build_program()
    return _PROG


def make_in_maps(x, W, U_k, V_k, lambda_k, v, router_W1, router_W2):
    bf = ml_dtypes.bfloat16
    f8e4 = ml_dtypes.float8_e4m3
    f8e5 = ml_dtypes.float8_e5m2
    x = np.asarray(x, dtype=np.float32)
    W = np.asarray(W, dtype=np.float32)
    U_k = np.asarray(U_k, dtype=np.float32)
    V_k = np.asarray(V_k, dtype=np.float32)
    lambda_k = np.asarray(lambda_k, dtype=np.float32)
    v = np.asarray(v, dtype=np.float32)
    router_W1 = np.asarray(router_W1, dtype=np.float32)
    router_W2 = np.asarray(router_W2, dtype=np.float32)

    scale = 1.0 + v                                       # (D,) per output row n
    wt = np.ascontiguousarray((W * scale[:, None]).T).astype(bf)     # (d, n)
    # ut8[p, kc, n] = (U*(1+v))[n, kc*128+p] in e5m2, natural scale
    ut = (U_k * scale[:, None]).T                                    # (k, n)
    ut8 = np.ascontiguousarray(
        ut.reshape(NK, P, D).transpose(1, 0, 2)).astype(f8e5)        # (P, NK, D)
    u1 = (U_k.astype(np.float64) @ router_W1.astype(np.float64)).astype(np.float32)
    lam = np.ascontiguousarray(lambda_k / 32.0).astype(bf)           # (E, K)
    w2 = router_W2.reshape(-1)
    # router max-shift folded in: logits' = (w2-a)*s1 - Relu((b-a)*s1)
    w2c = np.ascontiguousarray((w2 - w2.max()).reshape(1, E)).astype(bf)
    nab = np.array([[w2.min() - w2.max()]], dtype=np.float32)

    # c8[p, d2, :] = [ x[b, d2*128+p] | 32*V[k, d2*128+p] | u1[d2*128+p] pad ]
    v32 = (32.0 * V_k).T.reshape(ND, P, K).transpose(1, 0, 2)        # (P, ND, K)
    u1c = u1.reshape(ND, P).T[:, :, None]                            # (P, ND, 1)
    pad = np.zeros((P, ND, 15), dtype=np.float32)

    in_maps = []
    for c in range(N_CORES):
        xs = x[c * BS:(c + 1) * BS]                                  # (BS, D)
        xw = np.ascontiguousarray(
            np.concatenate([xs.T.astype(bf), wt], axis=1))           # (D, BS+D)
        x8 = xs.T.reshape(ND, P, BS).transpose(1, 0, 2)              # (P, ND, BS)
        c8 = np.ascontiguousarray(
            np.concatenate([x8, v32, u1c, pad], axis=2)).astype(f8e4)
        in_maps.append({"c8": c8, "xw": xw, "ut8": ut8,
                        "lam": lam, "w2c": w2c, "nab": nab})
    return in_maps


def run(in_maps, trace=False):
    nc = _get_prog()
    res = run_bass_kernel_spmd(nc, in_maps, core_ids=list(range(N_CORES)), trace=trace)
    out = np.concatenate(
        [res.results[c]["out"].astype(np.float32) for c in range(N_CORES)], axis=0)
    return out, res


def kernel(x, W, U_k, V_k, lambda_k, v, router_W1, router_W2):
    in_maps = make_in_maps(x, W, U_k, V_k, lambda_k, v, router_W1, router_W2)
    out, _ = run(in_maps, trace=False)
    return out
